# revision 1
# baseline (speedup 1.0000x reference)
"""LSTM encoder with EOS-freeze for Trainium2, data-parallel over batch on 8 cores.

Strategy
--------
Inputs are one-hot, so x @ Wi is a row-gather of Wi done with indirect DMA on
device. The recurrent h @ Wh runs on the tensor engine with Wh as 64 fp16
[128,128] stationary tiles (FWL) and h.T chunks as the [128,16] moving operand,
producing z transposed: PSUM [128 partitions = z-feature % 128, 16*tile + b].
Gates are reordered (i, f, o, g) host-side and the g block pre-scaled by 2 so a
single sigmoid over all 2048 gate columns yields tanh(g) = 2*sigmoid(2g) - 1.

The EOS freeze is handled without any per-step masking: sequences are
independent, so the kernel runs the unmasked recurrence and streams per-step
(c, h) snapshots to DRAM; the frozen value for sequence b is the snapshot at
its first-EOS step, selected during unshard.
"""

import numpy as np

try:
    import concourse  # noqa: F401
except ImportError:
    import sys

    sys.path.insert(0, "/opt/trn_rl_repo")

from contextlib import ExitStack

import concourse.bass as bass
import concourse.tile as tile
from concourse import bacc
from concourse import mybir
from concourse.bass import ds
from concourse.bass_utils import run_bass_kernel_spmd

dt = mybir.dt
Alu = mybir.AluOpType
Act = mybir.ActivationFunctionType

EOS_ID = 1
HID = 512
BATCH, SEQ, VOCAB = 128, 256, 1024
GATES = 4 * HID  # 2048
NCORES = 8
BLOC = BATCH // NCORES  # 16 sequences per core
NT = GATES // 128  # 16 feature tiles of z
NK = HID // 128  # 4 contraction chunks
BODY = 16  # steps per For_i iteration

# Collect profiling info when True (set by test.py; adds trace overhead).
TRACE = False
LAST_RESULTS = None  # BassKernelResults of the last run, for test.py

_PROGRAM = None


def _build_program(seq=SEQ, body=BODY):
    nc = bacc.Bacc("TRN2", debug=False, detect_race_conditions=False)

    wi = nc.declare_dram_parameter("wi", [VOCAB, GATES], dt.float16, isOutput=False)
    ident = nc.declare_dram_parameter("ident", [BLOC, BLOC], dt.float16, isOutput=False)
    wh = nc.declare_dram_parameter("wh", [128, NK * NT * 128], dt.float16, isOutput=False)
    tok = nc.declare_dram_parameter("tok", [BLOC, seq + body], dt.int32, isOutput=False)
    c_traj = nc.declare_dram_parameter("c_traj", [seq * 128, 64], dt.float32, isOutput=True)
    h_traj = nc.declare_dram_parameter("h_traj", [seq * 128, 64], dt.float16, isOutput=True)

    with tile.TileContext(nc) as tc, ExitStack() as ctx:
        pool = lambda name, bufs, **kw: ctx.enter_context(
            tc.tile_pool(name=name, bufs=bufs, **kw)
        )
        whp = pool("whp", 1)
        tokp = pool("tokp", 1)
        stp = pool("stp", 1)
        hp = pool("hp", 1)
        cp = pool("cp", 1)
        zp_pool = pool("zp", 2, space="PSUM")
        sp = pool("sp", 2)
        gp = pool("gp", 2)
        ap_ = pool("ap", 2)
        bp = pool("bp", 2)
        s2p = pool("s2p", 2)
        tp = pool("tp", 2)

        wh_sb = whp.tile([128, NK * NT * 128], dt.float16, name="wh_sb")
        nc.sync.dma_start(out=wh_sb[:], in_=wh[:, :])
        tok_cur = tokp.tile([BLOC, body], dt.int32, name="tok_cur")
        nc.sync.dma_start(out=tok_cur[:], in_=tok[:, 0:body])
        id_sb = tokp.tile([BLOC, BLOC], dt.float16, name="id_sb")
        nc.sync.dma_start(out=id_sb[:], in_=ident[:, :])

        ST = [stp.tile([BLOC, GATES], dt.float16, name=f"st{s}", tag=f"st{s}") for s in range(body)]
        H = [hp.tile([128, 64], dt.float16, name=f"h{s}", tag=f"h{s}") for s in range(body)]
        C = [cp.tile([128, 64], dt.float32, name=f"c{s}", tag=f"c{s}") for s in range(body)]

        nc.gpsimd.memset(H[body - 1][:], 0.0)
        nc.gpsimd.memset(C[body - 1][:], 0.0)
        for s in range(body):
            # init shadow coverage; real values come from the indirect gathers
            nc.gpsimd.memset(ST[s][:], 0.0)

        def gather_xp(s):
            # Gather BLOC wi rows (one per sequence) for one timestep into
            # ST[s][b, :] — row-per-partition, the DGE-supported shape.
            # tok_cur always holds the token column for the block being
            # prefetched, so the offset AP stays static.
            nc.gpsimd.indirect_dma_start(
                out=ST[s][:],
                out_offset=None,
                in_=wi[:, :],
                in_offset=bass.IndirectOffsetOnAxis(ap=tok_cur[:, s : s + 1], axis=0),
            )

        for s in range(body):
            gather_xp(s)

        def step(iv, s):
            hprev = H[(s - 1) % body]
            cprev = C[(s - 1) % body]
            zps = zp_pool.tile([128, 256], dt.float32, name="zps", tag="zpsum")
            # x@Wi enters PSUM via PE transpose of the gathered rows: these
            # matmuls need no h, so they overlap the previous step's tail.
            for t in range(NT):
                # start=True on the first matmul clears the bank's has_written
                # bits; every other matmul joins the same accumulation group.
                nc.tensor.matmul(
                    out=zps[:, 16 * t : 16 * t + 16],
                    lhsT=ST[s][:, 128 * t : 128 * t + 128],
                    rhs=id_sb[:],
                    start=(t == 0),
                    stop=False,
                )
            for k in range(NK):
                for t in range(NT):
                    nc.tensor.matmul(
                        out=zps[:, 16 * t : 16 * t + 16],
                        lhsT=wh_sb[:, (k * NT + t) * 128 : (k * NT + t) * 128 + 128],
                        rhs=hprev[:, 16 * k : 16 * k + 16],
                        start=False,
                        stop=(k == NK - 1 and t == NT - 1),
                    )
            S = sp.tile([128, 192], dt.float32, name="S", tag="S")
            nc.scalar.activation(out=S[:], in_=zps[:, 0:192], func=Act.Sigmoid)
            TG = gp.tile([128, 64], dt.float16, name="TG", tag="TG")
            nc.scalar.activation(out=TG[:], in_=zps[:, 192:256], func=Act.Tanh)
            A = ap_.tile([128, 64], dt.float32, name="A", tag="A")
            nc.vector.tensor_tensor(out=A[:], in0=S[:, 0:64], in1=TG[:], op=Alu.mult)
            B = bp.tile([128, 64], dt.float32, name="B", tag="B")
            nc.vector.tensor_tensor(out=B[:], in0=S[:, 64:128], in1=cprev[:], op=Alu.mult)
            cs = C[s]
            nc.vector.tensor_tensor(out=cs[:], in0=A[:], in1=B[:], op=Alu.add)
            T = tp.tile([128, 64], dt.float16, name="T", tag="T")
            nc.scalar.activation(out=T[:], in_=cs[:], func=Act.Tanh)
            hs = H[s]
            nc.vector.tensor_tensor(out=hs[:], in0=S[:, 128:192], in1=T[:], op=Alu.mult)

            nc.sync.dma_start(out=c_traj[ds((iv + s) * 128, 128), :], in_=cs[:])
            nc.sync.dma_start(out=h_traj[ds((iv + s) * 128, 128), :], in_=hs[:])
            # Prefetch this slot's xp for the next block (the token table is
            # padded so the final block reads harmless extra rows).
            gather_xp(s)

        with tc.For_i(0, seq, body, hint_engines=(mybir.EngineType.PE,), staggered_reset=True) as iv:
            # Stage the NEXT block's token columns; in-loop gathers prefetch
            # for block i+1 while this block computes.
            nc.sync.dma_start(out=tok_cur[:], in_=tok[:, ds(iv + body, body)])
            for s in range(body):
                step(iv, s)

    nc.finalize()
    return nc


def _get_program():
    global _PROGRAM
    if _PROGRAM is None:
        _PROGRAM = _build_program()
    return _PROGRAM


def _prep_host(inputs, Wi, Wh, b):
    tokens = np.argmax(inputs, axis=-1).astype(np.int32)  # [B, T]
    eos = inputs[:, :, EOS_ID] > 0.5
    any_eos = eos.any(axis=1)
    t_star = np.where(any_eos, eos.argmax(axis=1), SEQ - 1).astype(np.int64)

    # Gate reorder (i, f, o, g): one contiguous sigmoid over i,f,o and one
    # tanh over g (both live in the same ACT table set).
    perm = np.concatenate(
        [np.arange(0, 512), np.arange(512, 1024), np.arange(1536, 2048), np.arange(1024, 1536)]
    )
    Wi_re = (Wi.astype(np.float32) + b.astype(np.float32)[None, :])[:, perm]
    Wh_re = Wh.astype(np.float32)[:, perm]

    # wi rows stay in z-feature order (gate-permuted only); wh tile k*16+t
    # holds Wh_re[128k:128k+128, 128t:128t+128], stored partition-major.
    Wi_dev = np.ascontiguousarray(Wi_re).astype(np.float16)
    # Partition-major: wh[kr, (k*NT+t)*128 + p] = Wh_re[128k+kr, 128t+p]
    Wh_dev = np.ascontiguousarray(
        Wh_re.reshape(NK, 128, NT, 128).transpose(1, 0, 2, 3).reshape(128, NK * NT * 128)
    ).astype(np.float16)
    return tokens, t_star, Wi_dev, Wh_dev


def kernel(inputs, Wi, Wh, b):
    global LAST_RESULTS
    inputs = np.asarray(inputs)
    Wi = np.asarray(Wi)
    Wh = np.asarray(Wh)
    b = np.asarray(b)

    tokens, t_star, Wi_dev, Wh_dev = _prep_host(inputs, Wi, Wh, b)

    in_maps = []
    for n in range(NCORES):
        tokc = tokens[BLOC * n : BLOC * (n + 1)]
        tok_pad = np.concatenate([tokc, np.zeros((BLOC, BODY), np.int32)], axis=1)
        in_maps.append(
            {
                "wi": Wi_dev,
                "wh": Wh_dev,
                "tok": np.ascontiguousarray(tok_pad),
                "ident": np.eye(BLOC, dtype=np.float16),
            }
        )

    nc = _get_program()
    res = run_bass_kernel_spmd(nc, in_maps, list(range(NCORES)), trace=TRACE)
    LAST_RESULTS = res

    c_out = np.zeros((BATCH, HID), np.float32)
    h_out = np.zeros((BATCH, HID), np.float32)
    for n in range(NCORES):
        ct = res.results[n]["c_traj"].reshape(SEQ, 128, 64)
        ht = res.results[n]["h_traj"].reshape(SEQ, 128, 64).astype(np.float32)
        for bl in range(BLOC):
            g = BLOC * n + bl
            t = int(t_star[g])
            c_out[g] = ct[t][:, bl::BLOC].T.reshape(HID)
            h_out[g] = ht[t][:, bl::BLOC].T.reshape(HID)
    return (c_out, h_out)



# revision 5
# speedup vs baseline: 1.1879x; 1.1879x over previous
"""LSTM encoder with EOS-freeze for Trainium2, data-parallel over batch on 8 cores.

Strategy
--------
Inputs are one-hot, so x @ Wi is a row-gather of Wi done with indirect DMA on
device. The recurrent h @ Wh runs on the tensor engine with Wh as 64 fp16
[128,128] stationary tiles (FWL) and h.T chunks as the [128,16] moving operand,
producing z transposed: PSUM [128 partitions = z-feature % 128, 16*tile + b].
Gates are reordered (g, i, f, o) host-side, one PSUM bank per gate, so each
gate's activation starts as soon as its own 16 recurrent matmuls close instead
of waiting for all 64 — the elementwise chain overlaps the tensor engine.

The EOS freeze is handled without any per-step masking: sequences are
independent, so the kernel runs the unmasked recurrence and streams per-step
(c, h) snapshots to DRAM; the frozen value for sequence b is the snapshot at
its first-EOS step, selected during unshard.
"""

import numpy as np

try:
    import concourse  # noqa: F401
except ImportError:
    import sys

    sys.path.insert(0, "/opt/trn_rl_repo")

from contextlib import ExitStack

import concourse.bass as bass
import concourse.tile as tile
from concourse import bacc
from concourse import mybir
from concourse.bass import ds
from concourse.bass_utils import run_bass_kernel_spmd

dt = mybir.dt
Alu = mybir.AluOpType
Act = mybir.ActivationFunctionType

EOS_ID = 1
HID = 512
BATCH, SEQ, VOCAB = 128, 256, 1024
GATES = 4 * HID  # 2048
NCORES = 8
BLOC = BATCH // NCORES  # 16 sequences per core
NT = GATES // 128  # 16 feature tiles of z
NK = HID // 128  # 4 contraction chunks
BODY = 16  # steps per For_i iteration

# Collect profiling info when True (set by test.py; adds trace overhead).
TRACE = False
LAST_RESULTS = None  # BassKernelResults of the last run, for test.py

_PROGRAM = None


def _build_program(seq=SEQ, body=BODY):
    nc = bacc.Bacc("TRN2", debug=False, detect_race_conditions=False)

    wi = nc.declare_dram_parameter("wi", [VOCAB, GATES], dt.float16, isOutput=False)
    ident = nc.declare_dram_parameter("ident", [BLOC, BLOC], dt.float16, isOutput=False)
    wh = nc.declare_dram_parameter("wh", [128, NK * NT * 128], dt.float16, isOutput=False)
    tok = nc.declare_dram_parameter("tok", [BLOC, seq + body], dt.int32, isOutput=False)
    c_traj = nc.declare_dram_parameter("c_traj", [seq * 128, 64], dt.float32, isOutput=True)
    h_traj = nc.declare_dram_parameter("h_traj", [seq * 128, 64], dt.float16, isOutput=True)

    with tile.TileContext(nc) as tc, ExitStack() as ctx:
        pool = lambda name, bufs, **kw: ctx.enter_context(
            tc.tile_pool(name=name, bufs=bufs, **kw)
        )
        whp = pool("whp", 1)
        tokp = pool("tokp", 1)
        stp = pool("stp", 1)
        hp = pool("hp", 1)
        cp = pool("cp", 1)
        zp_pool = pool("zp", 2, space="PSUM")
        sp = pool("sp", 2)
        gp = pool("gp", 2)
        ap_ = pool("ap", 2)
        bp = pool("bp", 2)
        tp = pool("tp", 2)

        wh_sb = whp.tile([128, NK * NT * 128], dt.float16, name="wh_sb")
        nc.sync.dma_start(out=wh_sb[:], in_=wh[:, :])
        tok_cur = tokp.tile([BLOC, body], dt.int32, name="tok_cur")
        nc.sync.dma_start(out=tok_cur[:], in_=tok[:, 0:body])
        id_sb = tokp.tile([BLOC, BLOC], dt.float16, name="id_sb")
        nc.sync.dma_start(out=id_sb[:], in_=ident[:, :])

        ST = [stp.tile([BLOC, GATES], dt.float16, name=f"st{s}", tag=f"st{s}") for s in range(body)]
        H = [hp.tile([128, 64], dt.float16, name=f"h{s}", tag=f"h{s}") for s in range(body)]
        C = [cp.tile([128, 64], dt.float32, name=f"c{s}", tag=f"c{s}") for s in range(body)]

        nc.gpsimd.memset(H[body - 1][:], 0.0)
        nc.gpsimd.memset(C[body - 1][:], 0.0)
        for s in range(body):
            # init shadow coverage; real values come from the indirect gathers
            nc.gpsimd.memset(ST[s][:], 0.0)

        def gather_xp(s):
            # Gather BLOC wi rows (one per sequence) for one timestep into
            # ST[s][b, :] — row-per-partition, the DGE-supported shape.
            # tok_cur always holds the token column for the block being
            # prefetched, so the offset AP stays static.
            nc.gpsimd.indirect_dma_start(
                out=ST[s][:],
                out_offset=None,
                in_=wi[:, :],
                in_offset=bass.IndirectOffsetOnAxis(ap=tok_cur[:, s : s + 1], axis=0),
            )

        for s in range(body):
            gather_xp(s)

        def step(iv, s):
            hprev = H[(s - 1) % body]
            cprev = C[(s - 1) % body]
            # One PSUM bank per gate so each gate's accumulation group closes
            # after its own 16 matmuls and its activation overlaps the
            # remaining gates' matmuls. Gate order in z columns: g, i, f, o.
            Z = [
                zp_pool.tile([128, 64], dt.float32, name=f"z{q}", tag=f"z{q}")
                for q in range(4)
            ]
            # x@Wi enters PSUM via PE transpose of the gathered rows: these
            # matmuls need no h, so they overlap the previous step's tail.
            for q in range(4):
                for t in range(4):
                    tg = 4 * q + t
                    # start=True on the first matmul clears the bank's
                    # has_written bits; later matmuls join the group.
                    nc.tensor.matmul(
                        out=Z[q][:, 16 * t : 16 * t + 16],
                        lhsT=ST[s][:, 128 * tg : 128 * tg + 128],
                        rhs=id_sb[:],
                        start=(t == 0),
                        stop=False,
                    )
            for q in range(4):
                for t in range(4):
                    tg = 4 * q + t
                    for k in range(NK):
                        nc.tensor.matmul(
                            out=Z[q][:, 16 * t : 16 * t + 16],
                            lhsT=wh_sb[:, (k * NT + tg) * 128 : (k * NT + tg) * 128 + 128],
                            rhs=hprev[:, 16 * k : 16 * k + 16],
                            start=False,
                            stop=(t == 3 and k == NK - 1),
                        )
            TG = gp.tile([128, 64], dt.float16, name="TG", tag="TG")
            nc.scalar.activation(out=TG[:], in_=Z[0][:], func=Act.Tanh)
            SI = sp.tile([128, 64], dt.float32, name="SI", tag="SI")
            nc.scalar.activation(out=SI[:], in_=Z[1][:], func=Act.Sigmoid)
            SF = sp.tile([128, 64], dt.float32, name="SF", tag="SF")
            nc.scalar.activation(out=SF[:], in_=Z[2][:], func=Act.Sigmoid)
            SO = sp.tile([128, 64], dt.float32, name="SO", tag="SO")
            nc.scalar.activation(out=SO[:], in_=Z[3][:], func=Act.Sigmoid)
            A = ap_.tile([128, 64], dt.float32, name="A", tag="A")
            nc.vector.tensor_tensor(out=A[:], in0=SI[:], in1=TG[:], op=Alu.mult)
            B = bp.tile([128, 64], dt.float32, name="B", tag="B")
            nc.vector.tensor_tensor(out=B[:], in0=SF[:], in1=cprev[:], op=Alu.mult)
            cs = C[s]
            nc.vector.tensor_tensor(out=cs[:], in0=A[:], in1=B[:], op=Alu.add)
            T = tp.tile([128, 64], dt.float16, name="T", tag="T")
            nc.scalar.activation(out=T[:], in_=cs[:], func=Act.Tanh)
            hs = H[s]
            nc.vector.tensor_tensor(out=hs[:], in0=SO[:], in1=T[:], op=Alu.mult)

            nc.sync.dma_start(out=c_traj[ds((iv + s) * 128, 128), :], in_=cs[:])
            nc.sync.dma_start(out=h_traj[ds((iv + s) * 128, 128), :], in_=hs[:])
            # Prefetch this slot's xp for the next block (the token table is
            # padded so the final block reads harmless extra rows).
            gather_xp(s)

        with tc.For_i(0, seq, body, hint_engines=(mybir.EngineType.PE,), staggered_reset=True) as iv:
            # Stage the NEXT block's token columns; in-loop gathers prefetch
            # for block i+1 while this block computes.
            nc.sync.dma_start(out=tok_cur[:], in_=tok[:, ds(iv + body, body)])
            for s in range(body):
                step(iv, s)

    nc.finalize()
    return nc


def _get_program():
    global _PROGRAM
    if _PROGRAM is None:
        _PROGRAM = _build_program()
    return _PROGRAM


def _prep_host(inputs, Wi, Wh, b):
    tokens = np.argmax(inputs, axis=-1).astype(np.int32)  # [B, T]
    eos = inputs[:, :, EOS_ID] > 0.5
    any_eos = eos.any(axis=1)
    t_star = np.where(any_eos, eos.argmax(axis=1), SEQ - 1).astype(np.int64)

    # Gate reorder (g, i, f, o): gates whose results are needed earliest in
    # the elementwise chain close their PSUM banks first; o closes last and
    # has the shortest tail (h = sig(o) * tanh(c)).
    perm = np.concatenate(
        [np.arange(1024, 1536), np.arange(0, 512), np.arange(512, 1024), np.arange(1536, 2048)]
    )
    Wi_re = (Wi.astype(np.float32) + b.astype(np.float32)[None, :])[:, perm]
    Wh_re = Wh.astype(np.float32)[:, perm]

    # wi rows stay in z-feature order (gate-permuted only); wh tile k*16+t
    # holds Wh_re[128k:128k+128, 128t:128t+128], stored partition-major.
    Wi_dev = np.ascontiguousarray(Wi_re).astype(np.float16)
    # Partition-major: wh[kr, (k*NT+t)*128 + p] = Wh_re[128k+kr, 128t+p]
    Wh_dev = np.ascontiguousarray(
        Wh_re.reshape(NK, 128, NT, 128).transpose(1, 0, 2, 3).reshape(128, NK * NT * 128)
    ).astype(np.float16)
    return tokens, t_star, Wi_dev, Wh_dev


def kernel(inputs, Wi, Wh, b):
    global LAST_RESULTS
    inputs = np.asarray(inputs)
    Wi = np.asarray(Wi)
    Wh = np.asarray(Wh)
    b = np.asarray(b)

    tokens, t_star, Wi_dev, Wh_dev = _prep_host(inputs, Wi, Wh, b)

    in_maps = []
    for n in range(NCORES):
        tokc = tokens[BLOC * n : BLOC * (n + 1)]
        tok_pad = np.concatenate([tokc, np.zeros((BLOC, BODY), np.int32)], axis=1)
        in_maps.append(
            {
                "wi": Wi_dev,
                "wh": Wh_dev,
                "tok": np.ascontiguousarray(tok_pad),
                "ident": np.eye(BLOC, dtype=np.float16),
            }
        )

    nc = _get_program()
    res = run_bass_kernel_spmd(nc, in_maps, list(range(NCORES)), trace=TRACE)
    LAST_RESULTS = res

    c_out = np.zeros((BATCH, HID), np.float32)
    h_out = np.zeros((BATCH, HID), np.float32)
    for n in range(NCORES):
        ct = res.results[n]["c_traj"].reshape(SEQ, 128, 64)
        ht = res.results[n]["h_traj"].reshape(SEQ, 128, 64).astype(np.float32)
        for bl in range(BLOC):
            g = BLOC * n + bl
            t = int(t_star[g])
            c_out[g] = ct[t][:, bl::BLOC].T.reshape(HID)
            h_out[g] = ht[t][:, bl::BLOC].T.reshape(HID)
    return (c_out, h_out)



# revision 7
# speedup vs baseline: 1.2269x; 1.0328x over previous
"""LSTM encoder with EOS-freeze for Trainium2, data-parallel over batch on 8 cores.

Strategy
--------
Inputs are one-hot, so x @ Wi is a row-gather of Wi done with indirect DMA on
device. The recurrent h @ Wh runs on the tensor engine with Wh as 64 fp16
[128,128] stationary tiles (FWL) and h.T chunks as the [128,16] moving operand,
producing z transposed: PSUM [128 partitions = z-feature % 128, 16*tile + b].
Gates are reordered (g, i, f, o) host-side, one PSUM bank per gate, so each
gate's activation starts as soon as its own 16 recurrent matmuls close instead
of waiting for all 64 — the elementwise chain overlaps the tensor engine.

The EOS freeze is handled without any per-step masking: sequences are
independent, so the kernel runs the unmasked recurrence and streams per-step
(c, h) snapshots to DRAM; the frozen value for sequence b is the snapshot at
its first-EOS step, selected during unshard.
"""

import numpy as np

try:
    import concourse  # noqa: F401
except ImportError:
    import sys

    sys.path.insert(0, "/opt/trn_rl_repo")

from contextlib import ExitStack

import concourse.bass as bass
import concourse.tile as tile
from concourse import bacc
from concourse import mybir
from concourse.bass import ds
from concourse.bass_utils import run_bass_kernel_spmd

dt = mybir.dt
Alu = mybir.AluOpType
Act = mybir.ActivationFunctionType

EOS_ID = 1
HID = 512
BATCH, SEQ, VOCAB = 128, 256, 1024
GATES = 4 * HID  # 2048
NCORES = 8
BLOC = BATCH // NCORES  # 16 sequences per core
NT = GATES // 128  # 16 feature tiles of z
NK = HID // 128  # 4 contraction chunks
BODY = 32  # steps per For_i iteration

# Collect profiling info when True (set by test.py; adds trace overhead).
TRACE = False
LAST_RESULTS = None  # BassKernelResults of the last run, for test.py

_PROGRAM = None


def _build_program(seq=SEQ, body=BODY):
    nc = bacc.Bacc("TRN2", debug=False, detect_race_conditions=False)

    wi = nc.declare_dram_parameter("wi", [VOCAB, GATES], dt.float16, isOutput=False)
    ident = nc.declare_dram_parameter("ident", [BLOC, BLOC], dt.float16, isOutput=False)
    wh = nc.declare_dram_parameter("wh", [128, NK * NT * 128], dt.float16, isOutput=False)
    tok = nc.declare_dram_parameter("tok", [BLOC, seq + body], dt.int32, isOutput=False)
    c_traj = nc.declare_dram_parameter("c_traj", [seq * 128, 64], dt.float32, isOutput=True)
    h_traj = nc.declare_dram_parameter("h_traj", [seq * 128, 64], dt.float16, isOutput=True)

    with tile.TileContext(nc) as tc, ExitStack() as ctx:
        pool = lambda name, bufs, **kw: ctx.enter_context(
            tc.tile_pool(name=name, bufs=bufs, **kw)
        )
        whp = pool("whp", 1)
        tokp = pool("tokp", 1)
        stp = pool("stp", 1)
        hp = pool("hp", 1)
        cp = pool("cp", 1)
        zp_pool = pool("zp", 2, space="PSUM")
        sp = pool("sp", 2)
        gp = pool("gp", 2)
        ap_ = pool("ap", 2)
        bp = pool("bp", 2)
        tp = pool("tp", 2)

        wh_sb = whp.tile([128, NK * NT * 128], dt.float16, name="wh_sb")
        nc.sync.dma_start(out=wh_sb[:], in_=wh[:, :])
        tok_cur = tokp.tile([BLOC, body], dt.int32, name="tok_cur")
        nc.sync.dma_start(out=tok_cur[:], in_=tok[:, 0:body])
        id_sb = tokp.tile([BLOC, BLOC], dt.float16, name="id_sb")
        nc.sync.dma_start(out=id_sb[:], in_=ident[:, :])

        ST = [stp.tile([BLOC, GATES], dt.float16, name=f"st{s}", tag=f"st{s}") for s in range(body)]
        H = [hp.tile([128, 64], dt.float16, name=f"h{s}", tag=f"h{s}") for s in range(body)]
        C = [cp.tile([128, 64], dt.float32, name=f"c{s}", tag=f"c{s}") for s in range(body)]

        # Init on the vector engine: gpsimd memsets would serialize ahead of
        # the indirect-DMA gathers on the gpsimd queue and delay the first
        # block by ~30us.
        nc.vector.memset(H[body - 1][:], 0.0)
        nc.vector.memset(C[body - 1][:], 0.0)
        for s in range(body):
            # init shadow coverage; real values come from the indirect gathers
            nc.vector.memset(ST[s][:], 0.0)

        def gather_xp(s):
            # Gather BLOC wi rows (one per sequence) for one timestep into
            # ST[s][b, :] — row-per-partition, the DGE-supported shape.
            # tok_cur always holds the token column for the block being
            # prefetched, so the offset AP stays static.
            nc.gpsimd.indirect_dma_start(
                out=ST[s][:],
                out_offset=None,
                in_=wi[:, :],
                in_offset=bass.IndirectOffsetOnAxis(ap=tok_cur[:, s : s + 1], axis=0),
            )

        for s in range(body):
            gather_xp(s)

        def step(iv, s):
            hprev = H[(s - 1) % body]
            cprev = C[(s - 1) % body]
            # One PSUM bank per gate so each gate's accumulation group closes
            # after its own 16 matmuls and its activation overlaps the
            # remaining gates' matmuls. Gate order in z columns: g, i, f, o.
            Z = [
                zp_pool.tile([128, 64], dt.float32, name=f"z{q}", tag=f"z{q}")
                for q in range(4)
            ]
            # x@Wi enters PSUM via PE transpose of the gathered rows: these
            # matmuls need no h, so they overlap the previous step's tail.
            for q in range(4):
                for t in range(4):
                    tg = 4 * q + t
                    # start=True on the first matmul clears the bank's
                    # has_written bits; later matmuls join the group.
                    nc.tensor.matmul(
                        out=Z[q][:, 16 * t : 16 * t + 16],
                        lhsT=ST[s][:, 128 * tg : 128 * tg + 128],
                        rhs=id_sb[:],
                        start=(t == 0),
                        stop=False,
                    )
            for q in range(4):
                for t in range(4):
                    tg = 4 * q + t
                    for k in range(NK):
                        nc.tensor.matmul(
                            out=Z[q][:, 16 * t : 16 * t + 16],
                            lhsT=wh_sb[:, (k * NT + tg) * 128 : (k * NT + tg) * 128 + 128],
                            rhs=hprev[:, 16 * k : 16 * k + 16],
                            start=False,
                            stop=(t == 3 and k == NK - 1),
                        )
            TG = gp.tile([128, 64], dt.float16, name="TG", tag="TG")
            nc.scalar.activation(out=TG[:], in_=Z[0][:], func=Act.Tanh)
            SI = sp.tile([128, 64], dt.float32, name="SI", tag="SI")
            nc.scalar.activation(out=SI[:], in_=Z[1][:], func=Act.Sigmoid)
            SF = sp.tile([128, 64], dt.float32, name="SF", tag="SF")
            nc.scalar.activation(out=SF[:], in_=Z[2][:], func=Act.Sigmoid)
            SO = sp.tile([128, 64], dt.float32, name="SO", tag="SO")
            nc.scalar.activation(out=SO[:], in_=Z[3][:], func=Act.Sigmoid)
            A = ap_.tile([128, 64], dt.float32, name="A", tag="A")
            nc.vector.tensor_tensor(out=A[:], in0=SI[:], in1=TG[:], op=Alu.mult)
            B = bp.tile([128, 64], dt.float32, name="B", tag="B")
            nc.vector.tensor_tensor(out=B[:], in0=SF[:], in1=cprev[:], op=Alu.mult)
            cs = C[s]
            nc.vector.tensor_tensor(out=cs[:], in0=A[:], in1=B[:], op=Alu.add)
            T = tp.tile([128, 64], dt.float16, name="T", tag="T")
            nc.scalar.activation(out=T[:], in_=cs[:], func=Act.Tanh)
            hs = H[s]
            nc.vector.tensor_tensor(out=hs[:], in0=SO[:], in1=T[:], op=Alu.mult)

            nc.sync.dma_start(out=c_traj[ds((iv + s) * 128, 128), :], in_=cs[:])
            nc.sync.dma_start(out=h_traj[ds((iv + s) * 128, 128), :], in_=hs[:])
            # Prefetch this slot's xp for the next block (the token table is
            # padded so the final block reads harmless extra rows).
            gather_xp(s)

        with tc.For_i(0, seq, body, hint_engines=(mybir.EngineType.PE,), staggered_reset=True) as iv:
            # Stage the NEXT block's token columns; in-loop gathers prefetch
            # for block i+1 while this block computes.
            nc.sync.dma_start(out=tok_cur[:], in_=tok[:, ds(iv + body, body)])
            for s in range(body):
                step(iv, s)

    nc.finalize()
    return nc


def _get_program():
    global _PROGRAM
    if _PROGRAM is None:
        _PROGRAM = _build_program()
    return _PROGRAM


def _prep_host(inputs, Wi, Wh, b):
    tokens = np.argmax(inputs, axis=-1).astype(np.int32)  # [B, T]
    eos = inputs[:, :, EOS_ID] > 0.5
    any_eos = eos.any(axis=1)
    t_star = np.where(any_eos, eos.argmax(axis=1), SEQ - 1).astype(np.int64)

    # Gate reorder (g, i, f, o): gates whose results are needed earliest in
    # the elementwise chain close their PSUM banks first; o closes last and
    # has the shortest tail (h = sig(o) * tanh(c)).
    perm = np.concatenate(
        [np.arange(1024, 1536), np.arange(0, 512), np.arange(512, 1024), np.arange(1536, 2048)]
    )
    Wi_re = (Wi.astype(np.float32) + b.astype(np.float32)[None, :])[:, perm]
    Wh_re = Wh.astype(np.float32)[:, perm]

    # wi rows stay in z-feature order (gate-permuted only); wh tile k*16+t
    # holds Wh_re[128k:128k+128, 128t:128t+128], stored partition-major.
    Wi_dev = np.ascontiguousarray(Wi_re).astype(np.float16)
    # Partition-major: wh[kr, (k*NT+t)*128 + p] = Wh_re[128k+kr, 128t+p]
    Wh_dev = np.ascontiguousarray(
        Wh_re.reshape(NK, 128, NT, 128).transpose(1, 0, 2, 3).reshape(128, NK * NT * 128)
    ).astype(np.float16)
    return tokens, t_star, Wi_dev, Wh_dev


def kernel(inputs, Wi, Wh, b):
    global LAST_RESULTS
    inputs = np.asarray(inputs)
    Wi = np.asarray(Wi)
    Wh = np.asarray(Wh)
    b = np.asarray(b)

    tokens, t_star, Wi_dev, Wh_dev = _prep_host(inputs, Wi, Wh, b)

    in_maps = []
    for n in range(NCORES):
        tokc = tokens[BLOC * n : BLOC * (n + 1)]
        tok_pad = np.concatenate([tokc, np.zeros((BLOC, BODY), np.int32)], axis=1)
        in_maps.append(
            {
                "wi": Wi_dev,
                "wh": Wh_dev,
                "tok": np.ascontiguousarray(tok_pad),
                "ident": np.eye(BLOC, dtype=np.float16),
            }
        )

    nc = _get_program()
    res = run_bass_kernel_spmd(nc, in_maps, list(range(NCORES)), trace=TRACE)
    LAST_RESULTS = res

    c_out = np.zeros((BATCH, HID), np.float32)
    h_out = np.zeros((BATCH, HID), np.float32)
    for n in range(NCORES):
        ct = res.results[n]["c_traj"].reshape(SEQ, 128, 64)
        ht = res.results[n]["h_traj"].reshape(SEQ, 128, 64).astype(np.float32)
        for bl in range(BLOC):
            g = BLOC * n + bl
            t = int(t_star[g])
            c_out[g] = ct[t][:, bl::BLOC].T.reshape(HID)
            h_out[g] = ht[t][:, bl::BLOC].T.reshape(HID)
    return (c_out, h_out)



# revision 8
# speedup vs baseline: 1.2512x; 1.0198x over previous
"""LSTM encoder with EOS-freeze for Trainium2, data-parallel over batch on 8 cores.

Strategy
--------
Inputs are one-hot, so x @ Wi is a row-gather of Wi done with indirect DMA on
device. The recurrent h @ Wh runs on the tensor engine with Wh as 64 fp16
[128,128] stationary tiles (FWL) and h.T chunks as the [128,16] moving operand,
producing z transposed: PSUM [128 partitions = z-feature % 128, 16*tile + b].
Gates are reordered (g, i, f, o) host-side, one PSUM bank per gate, so each
gate's activation starts as soon as its own 16 recurrent matmuls close instead
of waiting for all 64 — the elementwise chain overlaps the tensor engine.

The EOS freeze is handled without any per-step masking: sequences are
independent, so the kernel runs the unmasked recurrence and streams per-step
(c, h) snapshots to DRAM; the frozen value for sequence b is the snapshot at
its first-EOS step, selected during unshard.
"""

import numpy as np

try:
    import concourse  # noqa: F401
except ImportError:
    import sys

    sys.path.insert(0, "/opt/trn_rl_repo")

from contextlib import ExitStack

import concourse.bass as bass
import concourse.tile as tile
from concourse import bacc
from concourse import mybir
from concourse.bass import ds
from concourse.bass_utils import run_bass_kernel_spmd

dt = mybir.dt
Alu = mybir.AluOpType
Act = mybir.ActivationFunctionType

EOS_ID = 1
HID = 512
BATCH, SEQ, VOCAB = 128, 256, 1024
GATES = 4 * HID  # 2048
NCORES = 8
BLOC = BATCH // NCORES  # 16 sequences per core
NT = GATES // 128  # 16 feature tiles of z
NK = HID // 128  # 4 contraction chunks
BODY = 32  # steps per For_i iteration

# Collect profiling info when True (set by test.py; adds trace overhead).
TRACE = False
LAST_RESULTS = None  # BassKernelResults of the last run, for test.py

_PROGRAM = None


def _build_program(seq=SEQ, body=BODY):
    nc = bacc.Bacc("TRN2", debug=False, detect_race_conditions=False)

    wi = nc.declare_dram_parameter("wi", [VOCAB, GATES], dt.float16, isOutput=False)
    ident = nc.declare_dram_parameter("ident", [BLOC, BLOC], dt.float16, isOutput=False)
    wh = nc.declare_dram_parameter("wh", [128, NK * NT * 128], dt.float16, isOutput=False)
    tok = nc.declare_dram_parameter("tok", [BLOC, seq + body], dt.int32, isOutput=False)
    c_traj = nc.declare_dram_parameter("c_traj", [seq * 128, 64], dt.float32, isOutput=True)
    h_traj = nc.declare_dram_parameter("h_traj", [seq * 128, 64], dt.float16, isOutput=True)

    with tile.TileContext(nc) as tc, ExitStack() as ctx:
        pool = lambda name, bufs, **kw: ctx.enter_context(
            tc.tile_pool(name=name, bufs=bufs, **kw)
        )
        whp = pool("whp", 1)
        tokp = pool("tokp", 1)
        stp = pool("stp", 1)
        hp = pool("hp", 1)
        cp = pool("cp", 1)
        zp_pool = pool("zp", 2, space="PSUM")
        sp = pool("sp", 2)
        gp = pool("gp", 2)
        ap_ = pool("ap", 2)
        bp = pool("bp", 2)
        tp = pool("tp", 2)

        wh_sb = whp.tile([128, NK * NT * 128], dt.float16, name="wh_sb")
        nc.sync.dma_start(out=wh_sb[:], in_=wh[:, :])
        tok_cur = tokp.tile([BLOC, body], dt.int32, name="tok_cur")
        nc.sync.dma_start(out=tok_cur[:], in_=tok[:, 0:body])
        id_sb = tokp.tile([BLOC, BLOC], dt.float16, name="id_sb")
        nc.sync.dma_start(out=id_sb[:], in_=ident[:, :])

        ST = [stp.tile([BLOC, GATES], dt.float16, name=f"st{s}", tag=f"st{s}") for s in range(body)]
        H = [hp.tile([128, 64], dt.float16, name=f"h{s}", tag=f"h{s}") for s in range(body)]
        C = [cp.tile([128, 64], dt.float32, name=f"c{s}", tag=f"c{s}") for s in range(body)]

        # Init on the vector engine: gpsimd memsets would serialize ahead of
        # the indirect-DMA gathers on the gpsimd queue and delay the first
        # block by ~30us. The ST tiles need no init: every gather writes the
        # full [BLOC, GATES] tile.
        nc.vector.memset(H[body - 1][:], 0.0)
        nc.vector.memset(C[body - 1][:], 0.0)

        def gather_xp(s):
            # Gather BLOC wi rows (one per sequence) for one timestep into
            # ST[s][b, :] — row-per-partition, the DGE-supported shape.
            # tok_cur always holds the token column for the block being
            # prefetched, so the offset AP stays static.
            nc.gpsimd.indirect_dma_start(
                out=ST[s][:],
                out_offset=None,
                in_=wi[:, :],
                in_offset=bass.IndirectOffsetOnAxis(ap=tok_cur[:, s : s + 1], axis=0),
            )

        for s in range(body):
            gather_xp(s)

        def step(iv, s):
            hprev = H[(s - 1) % body]
            cprev = C[(s - 1) % body]
            # One PSUM bank per gate so each gate's accumulation group closes
            # after its own 16 matmuls and its activation overlaps the
            # remaining gates' matmuls. Gate order in z columns: g, i, f, o.
            Z = [
                zp_pool.tile([128, 64], dt.float32, name=f"z{q}", tag=f"z{q}")
                for q in range(4)
            ]
            # x@Wi enters PSUM via PE transpose of the gathered rows: these
            # matmuls need no h, so they overlap the previous step's tail.
            for q in range(4):
                for t in range(4):
                    tg = 4 * q + t
                    # start=True on the first matmul clears the bank's
                    # has_written bits; later matmuls join the group.
                    nc.tensor.matmul(
                        out=Z[q][:, 16 * t : 16 * t + 16],
                        lhsT=ST[s][:, 128 * tg : 128 * tg + 128],
                        rhs=id_sb[:],
                        start=(t == 0),
                        stop=False,
                    )
            for q in range(4):
                for t in range(4):
                    tg = 4 * q + t
                    for k in range(NK):
                        nc.tensor.matmul(
                            out=Z[q][:, 16 * t : 16 * t + 16],
                            lhsT=wh_sb[:, (k * NT + tg) * 128 : (k * NT + tg) * 128 + 128],
                            rhs=hprev[:, 16 * k : 16 * k + 16],
                            start=False,
                            stop=(t == 3 and k == NK - 1),
                        )
            TG = gp.tile([128, 64], dt.float16, name="TG", tag="TG")
            nc.scalar.activation(out=TG[:], in_=Z[0][:], func=Act.Tanh)
            SI = sp.tile([128, 64], dt.float32, name="SI", tag="SI")
            nc.scalar.activation(out=SI[:], in_=Z[1][:], func=Act.Sigmoid)
            SF = sp.tile([128, 64], dt.float32, name="SF", tag="SF")
            nc.scalar.activation(out=SF[:], in_=Z[2][:], func=Act.Sigmoid)
            SO = sp.tile([128, 64], dt.float32, name="SO", tag="SO")
            nc.scalar.activation(out=SO[:], in_=Z[3][:], func=Act.Sigmoid)
            A = ap_.tile([128, 64], dt.float32, name="A", tag="A")
            nc.vector.tensor_tensor(out=A[:], in0=SI[:], in1=TG[:], op=Alu.mult)
            B = bp.tile([128, 64], dt.float32, name="B", tag="B")
            nc.vector.tensor_tensor(out=B[:], in0=SF[:], in1=cprev[:], op=Alu.mult)
            cs = C[s]
            nc.vector.tensor_tensor(out=cs[:], in0=A[:], in1=B[:], op=Alu.add)
            T = tp.tile([128, 64], dt.float16, name="T", tag="T")
            nc.scalar.activation(out=T[:], in_=cs[:], func=Act.Tanh)
            hs = H[s]
            nc.vector.tensor_tensor(out=hs[:], in0=SO[:], in1=T[:], op=Alu.mult)

            nc.sync.dma_start(out=c_traj[ds((iv + s) * 128, 128), :], in_=cs[:])
            nc.sync.dma_start(out=h_traj[ds((iv + s) * 128, 128), :], in_=hs[:])
            # Prefetch this slot's xp for the next block (the token table is
            # padded so the final block reads harmless extra rows).
            gather_xp(s)

        with tc.For_i(0, seq, body, hint_engines=(mybir.EngineType.PE,), staggered_reset=True) as iv:
            # Stage the NEXT block's token columns; in-loop gathers prefetch
            # for block i+1 while this block computes.
            nc.sync.dma_start(out=tok_cur[:], in_=tok[:, ds(iv + body, body)])
            for s in range(body):
                step(iv, s)

    nc.finalize()
    return nc


def _get_program():
    global _PROGRAM
    if _PROGRAM is None:
        _PROGRAM = _build_program()
    return _PROGRAM


def _prep_host(inputs, Wi, Wh, b):
    tokens = np.argmax(inputs, axis=-1).astype(np.int32)  # [B, T]
    eos = inputs[:, :, EOS_ID] > 0.5
    any_eos = eos.any(axis=1)
    t_star = np.where(any_eos, eos.argmax(axis=1), SEQ - 1).astype(np.int64)

    # Gate reorder (g, i, f, o): gates whose results are needed earliest in
    # the elementwise chain close their PSUM banks first; o closes last and
    # has the shortest tail (h = sig(o) * tanh(c)).
    perm = np.concatenate(
        [np.arange(1024, 1536), np.arange(0, 512), np.arange(512, 1024), np.arange(1536, 2048)]
    )
    Wi_re = (Wi.astype(np.float32) + b.astype(np.float32)[None, :])[:, perm]
    Wh_re = Wh.astype(np.float32)[:, perm]

    # wi rows stay in z-feature order (gate-permuted only); wh tile k*16+t
    # holds Wh_re[128k:128k+128, 128t:128t+128], stored partition-major.
    Wi_dev = np.ascontiguousarray(Wi_re).astype(np.float16)
    # Partition-major: wh[kr, (k*NT+t)*128 + p] = Wh_re[128k+kr, 128t+p]
    Wh_dev = np.ascontiguousarray(
        Wh_re.reshape(NK, 128, NT, 128).transpose(1, 0, 2, 3).reshape(128, NK * NT * 128)
    ).astype(np.float16)
    return tokens, t_star, Wi_dev, Wh_dev


def kernel(inputs, Wi, Wh, b):
    global LAST_RESULTS
    inputs = np.asarray(inputs)
    Wi = np.asarray(Wi)
    Wh = np.asarray(Wh)
    b = np.asarray(b)

    tokens, t_star, Wi_dev, Wh_dev = _prep_host(inputs, Wi, Wh, b)

    in_maps = []
    for n in range(NCORES):
        tokc = tokens[BLOC * n : BLOC * (n + 1)]
        tok_pad = np.concatenate([tokc, np.zeros((BLOC, BODY), np.int32)], axis=1)
        in_maps.append(
            {
                "wi": Wi_dev,
                "wh": Wh_dev,
                "tok": np.ascontiguousarray(tok_pad),
                "ident": np.eye(BLOC, dtype=np.float16),
            }
        )

    nc = _get_program()
    res = run_bass_kernel_spmd(nc, in_maps, list(range(NCORES)), trace=TRACE)
    LAST_RESULTS = res

    c_out = np.zeros((BATCH, HID), np.float32)
    h_out = np.zeros((BATCH, HID), np.float32)
    for n in range(NCORES):
        ct = res.results[n]["c_traj"].reshape(SEQ, 128, 64)
        ht = res.results[n]["h_traj"].reshape(SEQ, 128, 64).astype(np.float32)
        for bl in range(BLOC):
            g = BLOC * n + bl
            t = int(t_star[g])
            c_out[g] = ct[t][:, bl::BLOC].T.reshape(HID)
            h_out[g] = ht[t][:, bl::BLOC].T.reshape(HID)
    return (c_out, h_out)



# revision 12
# speedup vs baseline: 1.2918x; 1.0324x over previous
"""LSTM encoder with EOS-freeze for Trainium2, data-parallel over batch on 8 cores.

Strategy
--------
Inputs are one-hot, so x @ Wi is a row-gather of Wi done with indirect DMA on
device. The recurrent h @ Wh runs on the tensor engine with Wh as 64 fp16
[128,128] stationary tiles (FWL) and h.T chunks as the [128,16] moving operand,
producing z transposed: PSUM [128 partitions = z-feature % 128, 16*tile + b].
Gates are reordered (g, i, f, o) host-side, one PSUM bank per gate, so each
gate's activation starts as soon as its own 16 recurrent matmuls close instead
of waiting for all 64 — the elementwise chain overlaps the tensor engine.

The EOS freeze is handled without any per-step masking: sequences are
independent, so the kernel runs the unmasked recurrence and streams per-step
(c, h) snapshots to DRAM; the frozen value for sequence b is the snapshot at
its first-EOS step, selected during unshard.
"""

import numpy as np

try:
    import concourse  # noqa: F401
except ImportError:
    import sys

    sys.path.insert(0, "/opt/trn_rl_repo")

from contextlib import ExitStack

import concourse.bass as bass
import concourse.tile as tile
from concourse import bacc
from concourse import mybir
from concourse.bass import ds
from concourse.bass_utils import run_bass_kernel_spmd

dt = mybir.dt
Alu = mybir.AluOpType
Act = mybir.ActivationFunctionType

EOS_ID = 1
HID = 512
BATCH, SEQ, VOCAB = 128, 256, 1024
GATES = 4 * HID  # 2048
NCORES = 8
BLOC = BATCH // NCORES  # 16 sequences per core
NT = GATES // 128  # 16 feature tiles of z
NK = HID // 128  # 4 contraction chunks
BODY = 32  # steps per For_i iteration

# Collect profiling info when True (set by test.py; adds trace overhead).
TRACE = False
LAST_RESULTS = None  # BassKernelResults of the last run, for test.py

_PROGRAM = None


def _build_program(seq=SEQ, body=BODY):
    nc = bacc.Bacc("TRN2", debug=False, detect_race_conditions=False)

    wi = nc.declare_dram_parameter("wi", [VOCAB, GATES], dt.float16, isOutput=False)
    ident = nc.declare_dram_parameter("ident", [BLOC, BLOC], dt.float16, isOutput=False)
    wh = nc.declare_dram_parameter("wh", [128, NK * NT * 128], dt.float16, isOutput=False)
    tok = nc.declare_dram_parameter("tok", [BLOC, seq + body], dt.int32, isOutput=False)
    # Block 0's gathered Wi rows, prepared host-side: loading them with one
    # direct DMA keeps the gpsimd queue empty before the loop-entry barrier
    # (32 serial indirect gathers there used to cost ~45us of startup).
    st0 = nc.declare_dram_parameter("st0", [BLOC, body * GATES], dt.float16, isOutput=False)
    c_traj = nc.declare_dram_parameter("c_traj", [seq * 128, 64], dt.float32, isOutput=True)
    h_traj = nc.declare_dram_parameter("h_traj", [seq * 128, 64], dt.float16, isOutput=True)

    with tile.TileContext(nc) as tc, ExitStack() as ctx:
        pool = lambda name, bufs, **kw: ctx.enter_context(
            tc.tile_pool(name=name, bufs=bufs, **kw)
        )
        whp = pool("whp", 1)
        tokp = pool("tokp", 1)
        stp = pool("stp", 1)
        hp = pool("hp", 1)
        cp = pool("cp", 1)
        zp_pool = pool("zp", 2, space="PSUM")
        sp = pool("sp", 2)
        gp = pool("gp", 2)
        ap_ = pool("ap", 2)
        bp = pool("bp", 2)
        tp = pool("tp", 2)

        wh_sb = whp.tile([128, NK * NT * 128], dt.float16, name="wh_sb")
        nc.sync.dma_start(out=wh_sb[:], in_=wh[:, :])
        tok_cur = tokp.tile([BLOC, body], dt.int32, name="tok_cur")
        nc.sync.dma_start(out=tok_cur[:], in_=tok[:, 0:body])
        id_sb = tokp.tile([BLOC, BLOC], dt.float16, name="id_sb")
        nc.sync.dma_start(out=id_sb[:], in_=ident[:, :])

        # One big tile holding all `body` gathered-x slots; subtile dep
        # tracking scopes reads/writes to the per-step 2048-column slices.
        ST = stp.tile([BLOC, body * GATES], dt.float16, name="st_all")
        nc.sync.dma_start(out=ST[:], in_=st0[:, :])
        H = [hp.tile([128, 64], dt.float16, name=f"h{s}", tag=f"h{s}") for s in range(body)]
        C = [cp.tile([128, 64], dt.float32, name=f"c{s}", tag=f"c{s}") for s in range(body)]

        nc.vector.memset(H[body - 1][:], 0.0)
        nc.vector.memset(C[body - 1][:], 0.0)

        def gather_xp(s):
            # Gather BLOC wi rows (one per sequence) for one timestep into
            # ST[:, s-th slice] — row-per-partition, the DGE-supported shape.
            # tok_cur always holds the token column for the block being
            # prefetched, so the offset AP stays static.
            nc.gpsimd.indirect_dma_start(
                out=ST[:, s * GATES : (s + 1) * GATES],
                out_offset=None,
                in_=wi[:, :],
                in_offset=bass.IndirectOffsetOnAxis(ap=tok_cur[:, s : s + 1], axis=0),
            )

        def step(iv, s):
            hprev = H[(s - 1) % body]
            cprev = C[(s - 1) % body]
            # One PSUM bank per gate so each gate's accumulation group closes
            # after its own 16 matmuls and its activation overlaps the
            # remaining gates' matmuls. Gate order in z columns: g, i, f, o.
            Z = [
                zp_pool.tile([128, 64], dt.float32, name=f"z{q}", tag=f"z{q}")
                for q in range(4)
            ]
            # x@Wi enters PSUM via PE transpose of the gathered rows: these
            # matmuls need no h, so they overlap the previous step's tail.
            for q in range(4):
                for t in range(4):
                    tg = 4 * q + t
                    # start=True on the first matmul clears the bank's
                    # has_written bits; later matmuls join the group.
                    nc.tensor.matmul(
                        out=Z[q][:, 16 * t : 16 * t + 16],
                        lhsT=ST[:, s * GATES + 128 * tg : s * GATES + 128 * tg + 128],
                        rhs=id_sb[:],
                        start=(t == 0),
                        stop=False,
                    )
            for q in range(4):
                for t in range(4):
                    tg = 4 * q + t
                    for k in range(NK):
                        nc.tensor.matmul(
                            out=Z[q][:, 16 * t : 16 * t + 16],
                            lhsT=wh_sb[:, (k * NT + tg) * 128 : (k * NT + tg) * 128 + 128],
                            rhs=hprev[:, 16 * k : 16 * k + 16],
                            start=False,
                            stop=(t == 3 and k == NK - 1),
                        )
            TG = gp.tile([128, 64], dt.float16, name="TG", tag="TG")
            nc.scalar.activation(out=TG[:], in_=Z[0][:], func=Act.Tanh)
            SI = sp.tile([128, 64], dt.float32, name="SI", tag="SI")
            nc.scalar.activation(out=SI[:], in_=Z[1][:], func=Act.Sigmoid)
            SF = sp.tile([128, 64], dt.float32, name="SF", tag="SF")
            nc.scalar.activation(out=SF[:], in_=Z[2][:], func=Act.Sigmoid)
            SO = sp.tile([128, 64], dt.float32, name="SO", tag="SO")
            nc.scalar.activation(out=SO[:], in_=Z[3][:], func=Act.Sigmoid)
            A = ap_.tile([128, 64], dt.float32, name="A", tag="A")
            nc.vector.tensor_tensor(out=A[:], in0=SI[:], in1=TG[:], op=Alu.mult)
            B = bp.tile([128, 64], dt.float32, name="B", tag="B")
            nc.vector.tensor_tensor(out=B[:], in0=SF[:], in1=cprev[:], op=Alu.mult)
            cs = C[s]
            nc.vector.tensor_tensor(out=cs[:], in0=A[:], in1=B[:], op=Alu.add)
            T = tp.tile([128, 64], dt.float16, name="T", tag="T")
            nc.scalar.activation(out=T[:], in_=cs[:], func=Act.Tanh)
            hs = H[s]
            nc.vector.tensor_tensor(out=hs[:], in0=SO[:], in1=T[:], op=Alu.mult)

            nc.sync.dma_start(out=c_traj[ds((iv + s) * 128, 128), :], in_=cs[:])
            nc.sync.dma_start(out=h_traj[ds((iv + s) * 128, 128), :], in_=hs[:])
            # Prefetch this slot's xp for the next block (the token table is
            # padded so the final block reads harmless extra rows).
            gather_xp(s)

        with tc.For_i(0, seq, body, hint_engines=(mybir.EngineType.PE,), staggered_reset=True) as iv:
            # Stage the NEXT block's token columns; in-loop gathers prefetch
            # for block i+1 while this block computes.
            nc.sync.dma_start(out=tok_cur[:], in_=tok[:, ds(iv + body, body)])
            for s in range(body):
                step(iv, s)

    nc.finalize()
    return nc


def _get_program():
    global _PROGRAM
    if _PROGRAM is None:
        _PROGRAM = _build_program()
    return _PROGRAM


def _prep_host(inputs, Wi, Wh, b):
    tokens = np.argmax(inputs, axis=-1).astype(np.int32)  # [B, T]
    eos = inputs[:, :, EOS_ID] > 0.5
    any_eos = eos.any(axis=1)
    t_star = np.where(any_eos, eos.argmax(axis=1), SEQ - 1).astype(np.int64)

    # Gate reorder (g, i, f, o): gates whose results are needed earliest in
    # the elementwise chain close their PSUM banks first; o closes last and
    # has the shortest tail (h = sig(o) * tanh(c)).
    perm = np.concatenate(
        [np.arange(1024, 1536), np.arange(0, 512), np.arange(512, 1024), np.arange(1536, 2048)]
    )
    Wi_re = (Wi.astype(np.float32) + b.astype(np.float32)[None, :])[:, perm]
    Wh_re = Wh.astype(np.float32)[:, perm]

    # wi rows stay in z-feature order (gate-permuted only); wh tile k*16+t
    # holds Wh_re[128k:128k+128, 128t:128t+128], stored partition-major.
    Wi_dev = np.ascontiguousarray(Wi_re).astype(np.float16)
    # Partition-major: wh[kr, (k*NT+t)*128 + p] = Wh_re[128k+kr, 128t+p]
    Wh_dev = np.ascontiguousarray(
        Wh_re.reshape(NK, 128, NT, 128).transpose(1, 0, 2, 3).reshape(128, NK * NT * 128)
    ).astype(np.float16)
    return tokens, t_star, Wi_dev, Wh_dev


def kernel(inputs, Wi, Wh, b):
    global LAST_RESULTS
    inputs = np.asarray(inputs)
    Wi = np.asarray(Wi)
    Wh = np.asarray(Wh)
    b = np.asarray(b)

    tokens, t_star, Wi_dev, Wh_dev = _prep_host(inputs, Wi, Wh, b)

    in_maps = []
    for n in range(NCORES):
        tokc = tokens[BLOC * n : BLOC * (n + 1)]
        tok_pad = np.concatenate([tokc, np.zeros((BLOC, BODY), np.int32)], axis=1)
        st0 = Wi_dev[tokc[:, 0:BODY]].reshape(BLOC, BODY * GATES)
        in_maps.append(
            {
                "wi": Wi_dev,
                "wh": Wh_dev,
                "tok": np.ascontiguousarray(tok_pad),
                "ident": np.eye(BLOC, dtype=np.float16),
                "st0": np.ascontiguousarray(st0),
            }
        )

    nc = _get_program()
    res = run_bass_kernel_spmd(nc, in_maps, list(range(NCORES)), trace=TRACE)
    LAST_RESULTS = res

    c_out = np.zeros((BATCH, HID), np.float32)
    h_out = np.zeros((BATCH, HID), np.float32)
    for n in range(NCORES):
        ct = res.results[n]["c_traj"].reshape(SEQ, 128, 64)
        ht = res.results[n]["h_traj"].reshape(SEQ, 128, 64).astype(np.float32)
        for bl in range(BLOC):
            g = BLOC * n + bl
            t = int(t_star[g])
            c_out[g] = ct[t][:, bl::BLOC].T.reshape(HID)
            h_out[g] = ht[t][:, bl::BLOC].T.reshape(HID)
    return (c_out, h_out)



# revision 13
# speedup vs baseline: 1.3247x; 1.0255x over previous
"""LSTM encoder with EOS-freeze for Trainium2, data-parallel over batch on 8 cores.

Strategy
--------
Inputs are one-hot, so x @ Wi is a row-gather of Wi done with indirect DMA on
device. The recurrent h @ Wh runs on the tensor engine with Wh as 64 fp16
[128,128] stationary tiles and h.T chunks as the [128,16] moving operand,
producing z transposed: PSUM [128 partitions = z-feature % 128, 16*tile + b].
Gates are reordered (g, i, f, o) host-side, one PSUM bank per gate, so each
gate's activation starts as soon as its own 16 recurrent matmuls close instead
of waiting for all 64 — the elementwise chain overlaps the tensor engine.

Gathered x rows are packed 4 steps per 2048-column slice at partition offsets
{0,32,64,96} (the valid PE tile_position row bases for a 16-row stationary).
This spreads gather/preload DMAs across 64 partitions instead of 16 (4x DMA
bandwidth), shrinks SBUF so a 64-step loop body fits (3 For_i boundaries
instead of 15), and batches the per-block gathers into 16 DGE instructions so
the DMA queue is quiet at the loop boundary. Block 0's rows are gathered
host-side and loaded with one direct DMA so the gpsimd queue is empty before
the loop-entry barrier.

The EOS freeze is handled without any per-step masking: sequences are
independent, so the kernel runs the unmasked recurrence and streams per-step
(c, h) snapshots to DRAM; the frozen value for sequence b is the snapshot at
its first-EOS step, selected during unshard.
"""

import numpy as np

try:
    import concourse  # noqa: F401
except ImportError:
    import sys

    sys.path.insert(0, "/opt/trn_rl_repo")

from contextlib import ExitStack

import concourse.bass as bass
import concourse.tile as tile
from concourse import bacc
from concourse import mybir
from concourse.bass import ds
from concourse.bass_utils import run_bass_kernel_spmd

dt = mybir.dt
Alu = mybir.AluOpType
Act = mybir.ActivationFunctionType

EOS_ID = 1
HID = 512
BATCH, SEQ, VOCAB = 128, 256, 1024
GATES = 4 * HID  # 2048
NCORES = 8
BLOC = BATCH // NCORES  # 16 sequences per core
NT = GATES // 128  # 16 feature tiles of z
NK = HID // 128  # 4 contraction chunks
SPG = 4  # steps packed per ST column-slice (partition offsets 0/32/64/96)
BODY = 64  # steps per For_i iteration
NGRP = BODY // SPG  # gather groups per block

# Collect profiling info when True (set by test.py; adds trace overhead).
TRACE = False
LAST_RESULTS = None  # BassKernelResults of the last run, for test.py

_PROGRAM = None


def _build_program(seq=SEQ, body=BODY):
    ngrp = body // SPG
    nc = bacc.Bacc("TRN2", debug=False, detect_race_conditions=False)

    wi = nc.declare_dram_parameter("wi", [VOCAB, GATES], dt.float16, isOutput=False)
    ident = nc.declare_dram_parameter("ident", [128, BLOC], dt.float16, isOutput=False)
    wh = nc.declare_dram_parameter("wh", [128, NK * NT * 128], dt.float16, isOutput=False)
    # tok4[32u+b, j] = token of sequence b at step 4j+u (b<16; other rows 0).
    tok4 = nc.declare_dram_parameter(
        "tok4", [128, (seq + body) // SPG], dt.int32, isOutput=False
    )
    # Block 0's gathered Wi rows, prepared host-side (one direct DMA).
    st0 = nc.declare_dram_parameter("st0", [128, ngrp * GATES], dt.float16, isOutput=False)
    c_traj = nc.declare_dram_parameter("c_traj", [seq * 128, 64], dt.float32, isOutput=True)
    h_traj = nc.declare_dram_parameter("h_traj", [seq * 128, 64], dt.float16, isOutput=True)

    with tile.TileContext(nc) as tc, ExitStack() as ctx:
        pool = lambda name, bufs, **kw: ctx.enter_context(
            tc.tile_pool(name=name, bufs=bufs, **kw)
        )
        whp = pool("whp", 1)
        tokp = pool("tokp", 1)
        stp = pool("stp", 1)
        hp = pool("hp", 1)
        cp = pool("cp", 1)
        zp_pool = pool("zp", 2, space="PSUM")
        sp = pool("sp", 2)
        gp = pool("gp", 2)
        ap_ = pool("ap", 2)
        bp = pool("bp", 2)
        tp = pool("tp", 2)

        wh_sb = whp.tile([128, NK * NT * 128], dt.float16, name="wh_sb")
        nc.sync.dma_start(out=wh_sb[:], in_=wh[:, :])
        tok_cur = tokp.tile([128, ngrp], dt.int32, name="tok_cur")
        nc.sync.dma_start(out=tok_cur[:], in_=tok4[:, 0:ngrp])
        id_sb = tokp.tile([128, BLOC], dt.float16, name="id_sb")
        nc.sync.dma_start(out=id_sb[:], in_=ident[:, :])

        # One big tile holding all gather groups; subtile dep tracking scopes
        # reads/writes to the per-group 2048-column slices.
        ST = stp.tile([128, ngrp * GATES], dt.float16, name="st_all")
        nc.sync.dma_start(out=ST[:], in_=st0[:, :])
        H = [hp.tile([128, 64], dt.float16, name=f"h{s}", tag=f"h{s}") for s in range(body)]
        C = [cp.tile([128, 64], dt.float32, name=f"c{s}", tag=f"c{s}") for s in range(body)]

        nc.vector.memset(H[body - 1][:], 0.0)
        nc.vector.memset(C[body - 1][:], 0.0)

        def gather_xp(j):
            # Gather 128 wi rows (4 steps x 16 sequences, rows 16..31 of each
            # 32-partition group are dummy token 0) for group j — one row per
            # partition, the DGE-supported shape. tok_cur always holds the
            # token columns for the block being prefetched.
            nc.gpsimd.indirect_dma_start(
                out=ST[:, j * GATES : (j + 1) * GATES],
                out_offset=None,
                in_=wi[:, :],
                in_offset=bass.IndirectOffsetOnAxis(ap=tok_cur[:, j : j + 1], axis=0),
            )

        def step(jv, s):
            u, j = s % SPG, s // SPG
            hprev = H[(s - 1) % body]
            cprev = C[(s - 1) % body]
            # One PSUM bank per gate so each gate's accumulation group closes
            # after its own 16 matmuls and its activation overlaps the
            # remaining gates' matmuls. Gate order in z columns: g, i, f, o.
            Z = [
                zp_pool.tile([128, 64], dt.float32, name=f"z{q}", tag=f"z{q}")
                for q in range(4)
            ]
            # x@Wi enters PSUM via PE transpose of the gathered rows: these
            # matmuls need no h, so they overlap the previous step's tail.
            for q in range(4):
                for t in range(4):
                    tg = 4 * q + t
                    # start=True on the first matmul clears the bank's
                    # has_written bits; later matmuls join the group.
                    nc.tensor.matmul(
                        out=Z[q][:, 16 * t : 16 * t + 16],
                        lhsT=ST[
                            32 * u : 32 * u + BLOC,
                            j * GATES + 128 * tg : j * GATES + 128 * tg + 128,
                        ],
                        rhs=id_sb[32 * u : 32 * u + BLOC, :],
                        start=(t == 0),
                        stop=False,
                        tile_position=(32 * u, 0),
                    )
            for q in range(4):
                for t in range(4):
                    tg = 4 * q + t
                    for k in range(NK):
                        nc.tensor.matmul(
                            out=Z[q][:, 16 * t : 16 * t + 16],
                            lhsT=wh_sb[:, (k * NT + tg) * 128 : (k * NT + tg) * 128 + 128],
                            rhs=hprev[:, 16 * k : 16 * k + 16],
                            start=False,
                            stop=(t == 3 and k == NK - 1),
                        )
            TG = gp.tile([128, 64], dt.float16, name="TG", tag="TG")
            nc.scalar.activation(out=TG[:], in_=Z[0][:], func=Act.Tanh)
            SI = sp.tile([128, 64], dt.float32, name="SI", tag="SI")
            nc.scalar.activation(out=SI[:], in_=Z[1][:], func=Act.Sigmoid)
            SF = sp.tile([128, 64], dt.float32, name="SF", tag="SF")
            nc.scalar.activation(out=SF[:], in_=Z[2][:], func=Act.Sigmoid)
            SO = sp.tile([128, 64], dt.float32, name="SO", tag="SO")
            nc.scalar.activation(out=SO[:], in_=Z[3][:], func=Act.Sigmoid)
            A = ap_.tile([128, 64], dt.float32, name="A", tag="A")
            nc.vector.tensor_tensor(out=A[:], in0=SI[:], in1=TG[:], op=Alu.mult)
            B = bp.tile([128, 64], dt.float32, name="B", tag="B")
            nc.vector.tensor_tensor(out=B[:], in0=SF[:], in1=cprev[:], op=Alu.mult)
            cs = C[s]
            nc.vector.tensor_tensor(out=cs[:], in0=A[:], in1=B[:], op=Alu.add)
            T = tp.tile([128, 64], dt.float16, name="T", tag="T")
            nc.scalar.activation(out=T[:], in_=cs[:], func=Act.Tanh)
            hs = H[s]
            nc.vector.tensor_tensor(out=hs[:], in0=SO[:], in1=T[:], op=Alu.mult)

            nc.sync.dma_start(out=c_traj[ds((jv * SPG + s) * 128, 128), :], in_=cs[:])
            nc.sync.dma_start(out=h_traj[ds((jv * SPG + s) * 128, 128), :], in_=hs[:])
            # After the last step of group j has read its ST slice, prefetch
            # that slice for the next block (the token table is padded so the
            # final block reads harmless extra rows).
            if u == SPG - 1:
                gather_xp(j)

        with tc.For_i(
            0, seq // SPG, body // SPG, hint_engines=(mybir.EngineType.PE,), staggered_reset=True
        ) as jv:
            # Stage the NEXT block's token columns; in-loop gathers prefetch
            # for block i+1 while this block computes.
            nc.sync.dma_start(out=tok_cur[:], in_=tok4[:, ds(jv + ngrp, ngrp)])
            for s in range(body):
                step(jv, s)

    nc.finalize()
    return nc


def _get_program():
    global _PROGRAM
    if _PROGRAM is None:
        _PROGRAM = _build_program()
    return _PROGRAM


def _prep_host(inputs, Wi, Wh, b):
    tokens = np.argmax(inputs, axis=-1).astype(np.int32)  # [B, T]
    eos = inputs[:, :, EOS_ID] > 0.5
    any_eos = eos.any(axis=1)
    t_star = np.where(any_eos, eos.argmax(axis=1), SEQ - 1).astype(np.int64)

    # Gate reorder (g, i, f, o): gates whose results are needed earliest in
    # the elementwise chain close their PSUM banks first; o closes last and
    # has the shortest tail (h = sig(o) * tanh(c)).
    perm = np.concatenate(
        [np.arange(1024, 1536), np.arange(0, 512), np.arange(512, 1024), np.arange(1536, 2048)]
    )
    Wi_re = (Wi.astype(np.float32) + b.astype(np.float32)[None, :])[:, perm]
    Wh_re = Wh.astype(np.float32)[:, perm]

    # wi rows stay in z-feature order (gate-permuted only); wh tile k*16+t
    # holds Wh_re[128k:128k+128, 128t:128t+128], stored partition-major.
    Wi_dev = np.ascontiguousarray(Wi_re).astype(np.float16)
    # Partition-major: wh[kr, (k*NT+t)*128 + p] = Wh_re[128k+kr, 128t+p]
    Wh_dev = np.ascontiguousarray(
        Wh_re.reshape(NK, 128, NT, 128).transpose(1, 0, 2, 3).reshape(128, NK * NT * 128)
    ).astype(np.float16)
    return tokens, t_star, Wi_dev, Wh_dev


def _pack4(arr16):
    """[16, 4*n, ...] -> [128, n, ...] with step 4j+u at partition 32u+b."""
    n4 = arr16.shape[1]
    n = n4 // SPG
    out = np.zeros((128, n) + arr16.shape[2:], arr16.dtype)
    for u in range(SPG):
        out[32 * u : 32 * u + BLOC] = arr16[:, u::SPG]
    return out


def kernel(inputs, Wi, Wh, b):
    global LAST_RESULTS
    inputs = np.asarray(inputs)
    Wi = np.asarray(Wi)
    Wh = np.asarray(Wh)
    b = np.asarray(b)

    tokens, t_star, Wi_dev, Wh_dev = _prep_host(inputs, Wi, Wh, b)

    id_rep = np.zeros((128, BLOC), np.float16)
    for u in range(SPG):
        id_rep[32 * u : 32 * u + BLOC] = np.eye(BLOC, dtype=np.float16)

    in_maps = []
    for n in range(NCORES):
        tokc = tokens[BLOC * n : BLOC * (n + 1)]
        tok_pad = np.concatenate([tokc, np.zeros((BLOC, BODY), np.int32)], axis=1)
        tok4 = _pack4(tok_pad)
        st0 = _pack4(Wi_dev[tokc[:, 0:BODY]]).reshape(128, (BODY // SPG) * GATES)
        in_maps.append(
            {
                "wi": Wi_dev,
                "wh": Wh_dev,
                "tok4": np.ascontiguousarray(tok4),
                "ident": id_rep,
                "st0": np.ascontiguousarray(st0),
            }
        )

    nc = _get_program()
    res = run_bass_kernel_spmd(nc, in_maps, list(range(NCORES)), trace=TRACE)
    LAST_RESULTS = res

    c_out = np.zeros((BATCH, HID), np.float32)
    h_out = np.zeros((BATCH, HID), np.float32)
    for n in range(NCORES):
        ct = res.results[n]["c_traj"].reshape(SEQ, 128, 64)
        ht = res.results[n]["h_traj"].reshape(SEQ, 128, 64).astype(np.float32)
        for bl in range(BLOC):
            g = BLOC * n + bl
            t = int(t_star[g])
            c_out[g] = ct[t][:, bl::BLOC].T.reshape(HID)
            h_out[g] = ht[t][:, bl::BLOC].T.reshape(HID)
    return (c_out, h_out)
